# revision 9
# baseline (speedup 1.0000x reference)
"""Ewald reciprocal-space sum on 8 Trainium2 NeuronCores.

Math: for each system b, S(k) = sum_a q_a e^{i k.r_a} over the static
integer k-grid n in [-10,10]^3, k = n @ G, G = 2*pi*inv(cell)^T. With
phases phi_d = (r @ inv(cell))_d (turns), k.r = 2*pi*(n1*phi1 + n2*phi2
+ n3*phi3). Conjugate symmetry in the (n2,n3) plane means only the
rectangle n2 in [0,10] x n3 in [-10,10] (231 pairs) must be evaluated;
the n2<0 half is recovered on the host from the same partial sums.

Host precomputes (f64, cheap O(atoms) work):
  F2[a,j]  = frac-centered(j*phi2), j in 0..10
  F3[a,j]  = frac-centered(j*phi3), j in -10..10, and F3c = F3 - 1/4
  lhs[a]   = [q*cos(2*pi*j*phi1) | q*sin(...)] for j in 0..10  (22 cols)
Device work per core (SPMD, core c owns half the atoms of system c//2),
per 128-atom chunk t (8 chunks):
  V[b,j2,j3] = wrap(F2[j2] + F3ext[b,j3]) in [-1/2,1/2]   (1 fused DVE op)
  AA = Sin(-2*pi*V) -> [-sin(th23) | cos(th23)]           (1 ACT op)
  ps += lhs_t^T @ AA   [22 x 462] PSUM-accumulated        (1 PE matmul)
Host combines the 4 quadrant blocks of ps into S over all 441 pairs via
the mirror identity and applies the reference's k-space weights.
"""

import numpy as np

# ---- problem constants (hardcoded per contract) ----
B = 4
N_PER = 2000
NK = 10                      # k-grid extent: n in [-NK, NK]
NH = NK + 1                  # 11 non-negative values
NJ = 2 * NK + 1              # 21
NRECT = NH * NJ              # 231 pairs in the n2>=0 rectangle
DL = 2.0
SIGMA = 1.0
EPS = 1e-6
NORM = 90.0474
TWOPI = 2.0 * np.pi

N_CORES = 8
CORES_PER_SYS = 2
ATOMS_PER_CORE = (B * N_PER) // N_CORES     # 1000
CHUNKS = 8                                  # ceil(1000/128)
PADN = CHUNKS * 128                         # 1024

FTAB_COLS = CHUNKS * (NH + 2 * NJ)          # 424: F2 | F3 | F3c, t-major
LHS_COLS = CHUNKS * 2 * NH                  # 176
OUT_COLS = 2 * NRECT                        # 462

_CACHE = {}


def _build_nc():
    import concourse.bacc as bacc
    import concourse.bass as bass
    import concourse.bass_utils as bass_utils
    import concourse.mybir as mybir
    import concourse.tile as tile

    # Shrink the declared kernel semaphore pool: the NEFF postamble zeroes
    # every declared semaphore one EVENT_SEMAPHORE at a time (~90ns each),
    # so unused pool entries cost real exec time every launch.
    bass.get_kernel_semaphore_range = lambda: range(150, 184)

    # Cap walrus's own semaphore range for the same reason.
    if not getattr(bass_utils, "_max_sem_patched", False):
        _orig_walrus_args = bass_utils.get_walrus_args

        def _patched_walrus_args(*a, **k):
            return [*_orig_walrus_args(*a, **k), "--max-sem-num=80"]

        bass_utils.get_walrus_args = _patched_walrus_args
        bass_utils._max_sem_patched = True

    # cheaper TileContext exit: the Bass preamble re-clears the whole
    # kernel sem range at every execution, so the exit-time sem clear and
    # second all-engine barrier are redundant for this single-context
    # kernel; keep drain + one barrier.
    def _cheap_drain_and_barrier(self, tick_clock, wait_clock):
        drain_inst = self.nc.sync.drain()
        wait_clock.add_sem_waits(
            drain_inst.ins, tile.ScopedClock({None: tick_clock.global_clock})
        )
        popped = self.nc._tile_sem_poison_stack.pop()
        assert popped is self._sem_poison

    f32 = mybir.dt.float32
    f32r = mybir.dt.float32r
    Act = mybir.ActivationFunctionType

    # fused custom DVE op: out = wrap(in0 + in1 + s0) into [-s1, s1] with
    # period 1 (turn space)
    import concourse.dve_ops as dve_ops

    if not hasattr(dve_ops, "ADD_WRAP_EWALD"):
        from concourse.dve_spec import C0, C1, Spec, Src0, Src1, lower
        from concourse.dve_uop import DveOpSpec

        _y = (Src0 + Src1) + C0

        def _ref(in0, in1, s0, s1, imm2):
            y = in0 + in1 + s0
            return y + (
                (y < -s1).astype(np.float32) - (y > s1).astype(np.float32)
            )

        _spec = Spec(body=_y + ((_y < -C1) - (_y > C1)), reference=_ref)
        _shas = {
            ver: DveOpSpec(
                name="ADD_WRAP_EWALD", opcode=0,
                uops=lower(_spec, ver=ver), rd1_en=True,
            ).sha(ver)
            for ver in ("v3", "v4")
        }
        _op = dve_ops.DveOp("ADD_WRAP_EWALD", _spec, subdim=False, uops_sha=_shas)
        dve_ops.OPS.append(_op)
        dve_ops._SUB_OPCODE_FOR_NAME[_op.name] = (
            dve_ops._CUSTOM_DVE_ROW_BASE + len(dve_ops.OPS) - 1
        )
        dve_ops.CUSTOM_DVE_SPECS[_op.name] = _spec
        dve_ops.ADD_WRAP_EWALD = _op
    AW = dve_ops.ADD_WRAP_EWALD

    tile.TileContext._drain_and_barrier = _cheap_drain_and_barrier
    nc = bacc.Bacc(None, target_bir_lowering=False)

    ftab = nc.dram_tensor("ftab", [128, FTAB_COLS], f32, kind="ExternalInput")
    lhs = nc.dram_tensor("lhs", [128, LHS_COLS], f32r, kind="ExternalInput")
    sout = nc.dram_tensor("sout", [2 * NH, OUT_COLS], f32, kind="ExternalOutput")

    with tile.TileContext(nc) as tc:
        with (
            tc.tile_pool(name="const", bufs=1) as cp,
            tc.tile_pool(name="work", bufs=3) as wp,
            tc.tile_pool(name="psum", bufs=1, space="PSUM") as pp,
        ):
            ft = cp.tile([128, FTAB_COLS], f32)
            nc.sync.dma_start(out=ft[:], in_=ftab[:])
            lt = cp.tile([128, LHS_COLS], f32r)
            nc.gpsimd.dma_start(out=lt[:], in_=lhs[:])

            ps = pp.tile([2 * NH, OUT_COLS], f32)
            F3X_OFF = CHUNKS * NH                       # 88

            for t in range(CHUNKS):
                # pair col = bj*11 + j2, bj = b*21 + j3  (b=0: sin, b=1: cos)
                f2 = ft[:, NH * t : NH * (t + 1)]
                in0 = f2.unsqueeze(1).broadcast_to([128, 2 * NJ, NH])
                f3x = ft[:, F3X_OFF + 2 * NJ * t : F3X_OFF + 2 * NJ * (t + 1)]
                in1 = f3x.unsqueeze(2).broadcast_to([128, 2 * NJ, NH])
                V = wp.tile([128, OUT_COLS], f32)
                nc.vector._custom_dve(
                    AW,
                    out=V[:].rearrange("p (bj j2) -> p bj j2", j2=NH),
                    in0=in0, in1=in1, s0=0.0, s1=0.5,
                )
                AA = wp.tile([128, OUT_COLS], f32r)
                nc.scalar.activation(
                    out=AA[:], in_=V[:], func=Act.Sin, bias=0.0, scale=-TWOPI
                )
                nc.tensor.matmul(
                    out=ps[:],
                    lhsT=lt[:, 2 * NH * t : 2 * NH * (t + 1)],
                    rhs=AA[:],
                    start=(t == 0), stop=(t == CHUNKS - 1),
                )

            so = wp.tile([2 * NH, OUT_COLS], f32)
            nc.vector.tensor_copy(out=so[:, 0:NRECT], in_=ps[:, 0:NRECT])
            nc.scalar.activation(
                out=so[:, NRECT:OUT_COLS], in_=ps[:, NRECT:OUT_COLS],
                func=Act.Copy,
            )
            nc.sync.dma_start(out=sout[:], in_=so[:])

    nc.compile()
    return nc


def _get_nc():
    if "nc" not in _CACHE:
        _CACHE["nc"] = _build_nc()
    return _CACHE["nc"]


def _chunk_major(x, w):
    """atom a = t*128 + p  ->  [p, t*w + j]"""
    return x.reshape(CHUNKS, 128, w).transpose(1, 0, 2).reshape(128, CHUNKS * w)


def _host_inputs(q, r, cell):
    jj = np.arange(0, NH, dtype=np.float64)
    j3 = np.arange(-NK, NK + 1, dtype=np.float64)
    maps = []
    for c in range(N_CORES):
        b = c // CORES_PER_SYS
        half = c % CORES_PER_SYS
        lo = b * N_PER + half * ATOMS_PER_CORE
        rs = r[lo : lo + ATOMS_PER_CORE].astype(np.float64)
        qs = q[lo : lo + ATOMS_PER_CORE, 0].astype(np.float64)
        minv = np.linalg.inv(cell[b].astype(np.float64))
        phi = (rs @ minv) % 1.0
        phi_p = np.zeros((PADN, 3)); phi_p[:ATOMS_PER_CORE] = phi
        q_p = np.zeros(PADN); q_p[:ATOMS_PER_CORE] = qs
        ang1 = TWOPI * np.outer(phi_p[:, 0], jj)
        lhs = np.concatenate(
            [np.cos(ang1) * q_p[:, None], np.sin(ang1) * q_p[:, None]], axis=1
        )
        t2 = np.outer(phi_p[:, 1], jj); F2 = t2 - np.round(t2)
        t3 = np.outer(phi_p[:, 2], j3); F3 = t3 - np.round(t3)
        F3x = np.concatenate([F3, F3 - 0.25], axis=1)
        ftab = np.concatenate(
            [_chunk_major(F2, NH), _chunk_major(F3x, 2 * NJ)], axis=1
        ).astype(np.float32)
        maps.append({"ftab": ftab, "lhs": _chunk_major(lhs, 2 * NH).astype(np.float32)})
    return maps


def _host_weights(cell):
    """w[b, n1(0..10), n2(-10..10), n3(-10..10)] mirroring the reference."""
    k_sq_max = (TWOPI / DL) ** 2
    sigma_sq_half = SIGMA ** 2 / 2.0
    rng = np.arange(-NK, NK + 1, dtype=np.float64)
    n1, n2, n3 = np.meshgrid(rng[NK:], rng, rng, indexing="ij")
    nvec = np.stack([n1.ravel(), n2.ravel(), n3.ravel()], axis=1)
    hemi = (
        (nvec[:, 0] > 0)
        | ((nvec[:, 0] == 0) & (nvec[:, 1] > 0))
        | ((nvec[:, 0] == 0) & (nvec[:, 1] == 0) & (nvec[:, 2] > 0))
    )
    ws = []
    for b in range(B):
        cb = cell[b].astype(np.float64)
        G = TWOPI * np.linalg.inv(cb).T
        kvec = nvec @ G
        k_sq = np.sum(kvec ** 2, axis=1)
        mask = (k_sq > 0) & (k_sq <= k_sq_max) & hemi
        kfac = np.exp(-sigma_sq_half * k_sq) / (k_sq + EPS)
        vol = np.linalg.det(cb)
        ws.append(np.where(mask, 2.0 * kfac, 0.0) / vol)
    return np.stack(ws).reshape(B, NH, NJ, NJ)


def kernel(q, r, cell, batch):
    from concourse.bass_utils import run_bass_kernel_spmd

    q = np.asarray(q)
    r = np.asarray(r)
    cell = np.asarray(cell)

    nc = _get_nc()
    in_maps = _host_inputs(q, r, cell)
    res = run_bass_kernel_spmd(nc, in_maps, core_ids=list(range(N_CORES))).results

    w = _host_weights(cell)
    pot = np.zeros(B, np.float64)
    for b in range(B):
        M = (
            res[b * CORES_PER_SYS]["sout"].astype(np.float64)
            + res[b * CORES_PER_SYS + 1]["sout"].astype(np.float64)
        )
        # pair blocks are [n3, n2]-ordered: col = j3*11 + n2
        Crs = -M[0:NH, 0:NRECT].reshape(NH, NJ, NH)       # sum q c1 sin(th23)
        Css = -M[NH : 2 * NH, 0:NRECT].reshape(NH, NJ, NH)
        Crc = M[0:NH, NRECT:OUT_COLS].reshape(NH, NJ, NH)  # sum q c1 cos(th23)
        Csc = M[NH : 2 * NH, NRECT:OUT_COLS].reshape(NH, NJ, NH)
        wb = w[b]
        w_dir = wb[:, NK:, :].transpose(0, 2, 1)  # rect pair (n2, n3) itself
        w_mir = wb[:, NK::-1, ::-1].copy()        # its mirror (-n2, -n3)
        w_mir[:, 0, :] = 0.0                      # n2=0 row counted once
        w_mir = w_mir.transpose(0, 2, 1)
        s_sq_dir = (Crc - Css) ** 2 + (Crs + Csc) ** 2
        s_sq_mir = (Crc + Css) ** 2 + (Csc - Crs) ** 2
        recip = np.sum(w_dir * s_sq_dir) + np.sum(w_mir * s_sq_mir)
        qb = q[b * N_PER : (b + 1) * N_PER, 0].astype(np.float64)
        self_e = np.sum(qb ** 2) / (SIGMA * TWOPI ** 1.5)
        pot[b] = (recip - self_e) * NORM
    return pot.astype(np.float32)


# revision 12
# speedup vs baseline: 1.0478x; 1.0478x over previous
"""Ewald reciprocal-space sum on 8 Trainium2 NeuronCores.

Math: for each system b, S(k) = sum_a q_a e^{i k.r_a} over the static
integer k-grid n in [-10,10]^3, k = n @ G, G = 2*pi*inv(cell)^T. With
phases phi_d = (r @ inv(cell))_d (turns), k.r = 2*pi*(n1*phi1 + n2*phi2
+ n3*phi3). Conjugate symmetry in the (n2,n3) plane means only the
rectangle n2 in [0,10] x n3 in [-10,10] (231 pairs) must be evaluated;
the n2<0 half is recovered on the host from the same partial sums.

Host precomputes (f64, cheap O(atoms) work):
  F2[a,j]  = frac-centered(j*phi2), j in 0..10
  F3[a,j]  = frac-centered(j*phi3), j in -10..10, and F3c = F3 - 1/4
  lhs[a]   = [q*cos(2*pi*j*phi1) | q*sin(...)] for j in 0..10  (22 cols)
Device work per core (SPMD, core c owns half the atoms of system c//2),
per 128-atom chunk t (8 chunks):
  V[b,j2,j3] = wrap(F2[j2] + F3ext[b,j3]) in [-1/2,1/2]   (1 fused DVE op)
  AA = Sin(-2*pi*V) -> [-sin(th23) | cos(th23)]           (1 ACT op)
  ps += lhs_t^T @ AA   [22 x 462] PSUM-accumulated        (1 PE matmul)
Host combines the 4 quadrant blocks of ps into S over all 441 pairs via
the mirror identity and applies the reference's k-space weights.
"""

import numpy as np

# ---- problem constants (hardcoded per contract) ----
B = 4
N_PER = 2000
NK = 10                      # k-grid extent: n in [-NK, NK]
NH = NK + 1                  # 11 non-negative values
NJ = 2 * NK + 1              # 21
NRECT = NH * NJ              # 231 pairs in the n2>=0 rectangle
DL = 2.0
SIGMA = 1.0
EPS = 1e-6
NORM = 90.0474
TWOPI = 2.0 * np.pi

N_CORES = 8
CORES_PER_SYS = 2
ATOMS_PER_CORE = (B * N_PER) // N_CORES     # 1000
CHUNKS = 8                                  # ceil(1000/128)
PADN = CHUNKS * 128                         # 1024

FTAB_COLS = CHUNKS * (NH + 2 * NJ)          # 424: F2 | F3 | F3c, t-major
LHS_COLS = CHUNKS * 2 * NH                  # 176
OUT_COLS = 2 * NRECT                        # 462

_CACHE = {}


def _build_nc():
    import concourse.bacc as bacc
    import concourse.mybir as mybir
    import concourse.tile as tile

    # cheaper TileContext exit: the Bass preamble re-clears the whole
    # kernel sem range at every execution, so the exit-time sem clear and
    # second all-engine barrier are redundant for this single-context
    # kernel; keep drain + one barrier.
    def _cheap_drain_and_barrier(self, tick_clock, wait_clock):
        drain_inst = self.nc.sync.drain()
        wait_clock.add_sem_waits(
            drain_inst.ins, tile.ScopedClock({None: tick_clock.global_clock})
        )
        popped = self.nc._tile_sem_poison_stack.pop()
        assert popped is self._sem_poison

    f32 = mybir.dt.float32
    f32r = mybir.dt.float32r
    Act = mybir.ActivationFunctionType

    # fused custom DVE op: out = wrap(in0 + in1 + s0) into [-s1, s1] with
    # period 1 (turn space)
    import concourse.dve_ops as dve_ops

    if not hasattr(dve_ops, "ADD_WRAP_EWALD"):
        from concourse.dve_spec import C0, C1, Spec, Src0, Src1, lower
        from concourse.dve_uop import DveOpSpec

        _y = (Src0 + Src1) + C0

        def _ref(in0, in1, s0, s1, imm2):
            y = in0 + in1 + s0
            return y + (
                (y < -s1).astype(np.float32) - (y > s1).astype(np.float32)
            )

        _spec = Spec(body=_y + ((_y < -C1) - (_y > C1)), reference=_ref)
        _shas = {
            ver: DveOpSpec(
                name="ADD_WRAP_EWALD", opcode=0,
                uops=lower(_spec, ver=ver), rd1_en=True,
            ).sha(ver)
            for ver in ("v3", "v4")
        }
        _op = dve_ops.DveOp("ADD_WRAP_EWALD", _spec, subdim=False, uops_sha=_shas)
        dve_ops.OPS.append(_op)
        dve_ops._SUB_OPCODE_FOR_NAME[_op.name] = (
            dve_ops._CUSTOM_DVE_ROW_BASE + len(dve_ops.OPS) - 1
        )
        dve_ops.CUSTOM_DVE_SPECS[_op.name] = _spec
        dve_ops.ADD_WRAP_EWALD = _op
    AW = dve_ops.ADD_WRAP_EWALD

    tile.TileContext._drain_and_barrier = _cheap_drain_and_barrier
    nc = bacc.Bacc(None, target_bir_lowering=False)

    ftab = nc.dram_tensor("ftab", [128, FTAB_COLS], f32, kind="ExternalInput")
    lhs = nc.dram_tensor("lhs", [128, LHS_COLS], f32r, kind="ExternalInput")
    sout = nc.dram_tensor("sout", [2 * NH, OUT_COLS], f32, kind="ExternalOutput")

    with tile.TileContext(nc) as tc:
        CC = NH + 2 * NJ                                # 53 cols per chunk
        with (
            tc.tile_pool(name="const", bufs=1) as cp,
            tc.tile_pool(name="work", bufs=3) as wp,
            tc.tile_pool(name="psum", bufs=1, space="PSUM") as pp,
        ):
            ft = cp.tile([128, FTAB_COLS], f32)
            # split so chunks 0-1 land (and unblock the DVE) early; both
            # sync and scalar front-ends are HWDGE rings, so the two
            # pieces stream in parallel.
            nc.sync.dma_start(out=ft[:, 0 : 2 * CC], in_=ftab[:, 0 : 2 * CC])
            nc.scalar.dma_start(
                out=ft[:, 2 * CC : FTAB_COLS], in_=ftab[:, 2 * CC : FTAB_COLS]
            )
            lt = cp.tile([128, LHS_COLS], f32r)
            nc.gpsimd.dma_start(out=lt[:], in_=lhs[:])

            ps = pp.tile([2 * NH, OUT_COLS], f32)

            for u in range(CHUNKS // 2):
                # two chunks share one V/AA tile so one ACT op covers both
                V = wp.tile([128, 2 * OUT_COLS], f32)
                for h in range(2):
                    t = 2 * u + h
                    # pair col = bj*11 + j2, bj = b*21 + j3 (b=0 sin, b=1 cos)
                    f2 = ft[:, CC * t : CC * t + NH]
                    in0 = f2.unsqueeze(1).broadcast_to([128, 2 * NJ, NH])
                    f3x = ft[:, CC * t + NH : CC * (t + 1)]
                    in1 = f3x.unsqueeze(2).broadcast_to([128, 2 * NJ, NH])
                    nc.vector._custom_dve(
                        AW,
                        out=V[:, OUT_COLS * h : OUT_COLS * (h + 1)].rearrange(
                            "p (bj j2) -> p bj j2", j2=NH
                        ),
                        in0=in0, in1=in1, s0=0.0, s1=0.5,
                    )
                AA = wp.tile([128, 2 * OUT_COLS], f32r)
                nc.scalar.activation(
                    out=AA[:], in_=V[:], func=Act.Sin, bias=0.0, scale=-TWOPI
                )
                for h in range(2):
                    t = 2 * u + h
                    nc.tensor.matmul(
                        out=ps[:],
                        lhsT=lt[:, 2 * NH * t : 2 * NH * (t + 1)],
                        rhs=AA[:, OUT_COLS * h : OUT_COLS * (h + 1)],
                        start=(t == 0), stop=(t == CHUNKS - 1),
                    )

            so = wp.tile([2 * NH, OUT_COLS], f32)
            nc.vector.tensor_copy(out=so[:], in_=ps[:])
            nc.sync.dma_start(out=sout[:], in_=so[:])

    nc.compile()
    return nc


def _get_nc():
    if "nc" not in _CACHE:
        _CACHE["nc"] = _build_nc()
    return _CACHE["nc"]


def _chunk_major(x, w):
    """atom a = t*128 + p  ->  [p, t*w + j]"""
    return x.reshape(CHUNKS, 128, w).transpose(1, 0, 2).reshape(128, CHUNKS * w)


def _host_inputs(q, r, cell):
    jj = np.arange(0, NH, dtype=np.float64)
    j3 = np.arange(-NK, NK + 1, dtype=np.float64)
    maps = []
    for c in range(N_CORES):
        b = c // CORES_PER_SYS
        half = c % CORES_PER_SYS
        lo = b * N_PER + half * ATOMS_PER_CORE
        rs = r[lo : lo + ATOMS_PER_CORE].astype(np.float64)
        qs = q[lo : lo + ATOMS_PER_CORE, 0].astype(np.float64)
        minv = np.linalg.inv(cell[b].astype(np.float64))
        phi = (rs @ minv) % 1.0
        phi_p = np.zeros((PADN, 3)); phi_p[:ATOMS_PER_CORE] = phi
        q_p = np.zeros(PADN); q_p[:ATOMS_PER_CORE] = qs
        ang1 = TWOPI * np.outer(phi_p[:, 0], jj)
        lhs = np.concatenate(
            [np.cos(ang1) * q_p[:, None], np.sin(ang1) * q_p[:, None]], axis=1
        )
        t2 = np.outer(phi_p[:, 1], jj); F2 = t2 - np.round(t2)
        t3 = np.outer(phi_p[:, 2], j3); F3 = t3 - np.round(t3)
        # per-chunk groups of 53 cols: F2 (11) | F3 (21) | F3 - 1/4 (21)
        ftab = _chunk_major(
            np.concatenate([F2, F3, F3 - 0.25], axis=1), NH + 2 * NJ
        ).astype(np.float32)
        maps.append({"ftab": ftab, "lhs": _chunk_major(lhs, 2 * NH).astype(np.float32)})
    return maps


def _host_weights(cell):
    """w[b, n1(0..10), n2(-10..10), n3(-10..10)] mirroring the reference."""
    k_sq_max = (TWOPI / DL) ** 2
    sigma_sq_half = SIGMA ** 2 / 2.0
    rng = np.arange(-NK, NK + 1, dtype=np.float64)
    n1, n2, n3 = np.meshgrid(rng[NK:], rng, rng, indexing="ij")
    nvec = np.stack([n1.ravel(), n2.ravel(), n3.ravel()], axis=1)
    hemi = (
        (nvec[:, 0] > 0)
        | ((nvec[:, 0] == 0) & (nvec[:, 1] > 0))
        | ((nvec[:, 0] == 0) & (nvec[:, 1] == 0) & (nvec[:, 2] > 0))
    )
    ws = []
    for b in range(B):
        cb = cell[b].astype(np.float64)
        G = TWOPI * np.linalg.inv(cb).T
        kvec = nvec @ G
        k_sq = np.sum(kvec ** 2, axis=1)
        mask = (k_sq > 0) & (k_sq <= k_sq_max) & hemi
        kfac = np.exp(-sigma_sq_half * k_sq) / (k_sq + EPS)
        vol = np.linalg.det(cb)
        ws.append(np.where(mask, 2.0 * kfac, 0.0) / vol)
    return np.stack(ws).reshape(B, NH, NJ, NJ)


def kernel(q, r, cell, batch):
    from concourse.bass_utils import run_bass_kernel_spmd

    q = np.asarray(q)
    r = np.asarray(r)
    cell = np.asarray(cell)

    nc = _get_nc()
    in_maps = _host_inputs(q, r, cell)
    res = run_bass_kernel_spmd(nc, in_maps, core_ids=list(range(N_CORES))).results

    w = _host_weights(cell)
    pot = np.zeros(B, np.float64)
    for b in range(B):
        M = (
            res[b * CORES_PER_SYS]["sout"].astype(np.float64)
            + res[b * CORES_PER_SYS + 1]["sout"].astype(np.float64)
        )
        # pair blocks are [n3, n2]-ordered: col = j3*11 + n2
        Crs = -M[0:NH, 0:NRECT].reshape(NH, NJ, NH)       # sum q c1 sin(th23)
        Css = -M[NH : 2 * NH, 0:NRECT].reshape(NH, NJ, NH)
        Crc = M[0:NH, NRECT:OUT_COLS].reshape(NH, NJ, NH)  # sum q c1 cos(th23)
        Csc = M[NH : 2 * NH, NRECT:OUT_COLS].reshape(NH, NJ, NH)
        wb = w[b]
        w_dir = wb[:, NK:, :].transpose(0, 2, 1)  # rect pair (n2, n3) itself
        w_mir = wb[:, NK::-1, ::-1].copy()        # its mirror (-n2, -n3)
        w_mir[:, 0, :] = 0.0                      # n2=0 row counted once
        w_mir = w_mir.transpose(0, 2, 1)
        s_sq_dir = (Crc - Css) ** 2 + (Crs + Csc) ** 2
        s_sq_mir = (Crc + Css) ** 2 + (Csc - Crs) ** 2
        recip = np.sum(w_dir * s_sq_dir) + np.sum(w_mir * s_sq_mir)
        qb = q[b * N_PER : (b + 1) * N_PER, 0].astype(np.float64)
        self_e = np.sum(qb ** 2) / (SIGMA * TWOPI ** 1.5)
        pot[b] = (recip - self_e) * NORM
    return pot.astype(np.float32)


# revision 13
# speedup vs baseline: 1.0867x; 1.0372x over previous
"""Ewald reciprocal-space sum on 8 Trainium2 NeuronCores.

Math: for each system b, S(k) = sum_a q_a e^{i k.r_a} over the static
integer k-grid n in [-10,10]^3, k = n @ G, G = 2*pi*inv(cell)^T. With
phases phi_d = (r @ inv(cell))_d (turns), k.r = 2*pi*(n1*phi1 + n2*phi2
+ n3*phi3). Conjugate symmetry in the (n2,n3) plane means only the
rectangle n2 in [0,10] x n3 in [-10,10] (231 pairs) must be evaluated;
the n2<0 half is recovered on the host from the same partial sums.

Host precomputes (f64, cheap O(atoms) work):
  F2[a,j]  = frac-centered(j*phi2), j in 0..10
  F3[a,j]  = frac-centered(j*phi3), j in -10..10, and F3c = F3 - 1/4
  lhs[a]   = [q*cos(2*pi*j*phi1) | q*sin(...)] for j in 0..10  (22 cols)
Device work per core (SPMD, core c owns half the atoms of system c//2),
per 128-atom chunk t (8 chunks):
  V[b,j2,j3] = wrap(F2[j2] + F3ext[b,j3]) in [-1/2,1/2]   (1 fused DVE op)
  AA = Sin(-2*pi*V) -> [-sin(th23) | cos(th23)]           (1 ACT op)
  ps += lhs_t^T @ AA   [22 x 462] PSUM-accumulated        (1 PE matmul)
Host combines the 4 quadrant blocks of ps into S over all 441 pairs via
the mirror identity and applies the reference's k-space weights.
"""

import numpy as np

# ---- problem constants (hardcoded per contract) ----
B = 4
N_PER = 2000
NK = 10                      # k-grid extent: n in [-NK, NK]
NH = NK + 1                  # 11 non-negative values
NJ = 2 * NK + 1              # 21
NRECT = NH * NJ              # 231 pairs in the n2>=0 rectangle
DL = 2.0
SIGMA = 1.0
EPS = 1e-6
NORM = 90.0474
TWOPI = 2.0 * np.pi

N_CORES = 8
CORES_PER_SYS = 2
ATOMS_PER_CORE = (B * N_PER) // N_CORES     # 1000
CHUNKS = 8                                  # ceil(1000/128)
PADN = CHUNKS * 128                         # 1024

FTAB_COLS = CHUNKS * (NH + 2 * NJ)          # 424: F2 | F3 | F3c, t-major
LHS_COLS = CHUNKS * 2 * NH                  # 176
OUT_COLS = 2 * NRECT                        # 462

_CACHE = {}


def _build_nc():
    import concourse.bacc as bacc
    import concourse.mybir as mybir
    import concourse.tile as tile

    # cheaper TileContext exit: the Bass preamble re-clears the whole
    # kernel sem range at every execution, so the exit-time sem clear and
    # second all-engine barrier are redundant for this single-context
    # kernel; keep drain + one barrier.
    def _cheap_drain_and_barrier(self, tick_clock, wait_clock):
        drain_inst = self.nc.sync.drain()
        wait_clock.add_sem_waits(
            drain_inst.ins, tile.ScopedClock({None: tick_clock.global_clock})
        )
        popped = self.nc._tile_sem_poison_stack.pop()
        assert popped is self._sem_poison

    f32 = mybir.dt.float32
    f32r = mybir.dt.float32r
    Act = mybir.ActivationFunctionType

    # fused custom DVE op: out = wrap(in0 + in1 + s0) into [-s1, s1] with
    # period 1 (turn space)
    import concourse.dve_ops as dve_ops

    if not hasattr(dve_ops, "ADD_WRAP_EWALD"):
        from concourse.dve_spec import C0, C1, Spec, Src0, Src1, lower
        from concourse.dve_uop import DveOpSpec

        _y = (Src0 + Src1) + C0

        def _ref(in0, in1, s0, s1, imm2):
            y = in0 + in1 + s0
            return y + (
                (y < -s1).astype(np.float32) - (y > s1).astype(np.float32)
            )

        _spec = Spec(body=_y + ((_y < -C1) - (_y > C1)), reference=_ref)
        _shas = {
            ver: DveOpSpec(
                name="ADD_WRAP_EWALD", opcode=0,
                uops=lower(_spec, ver=ver), rd1_en=True,
            ).sha(ver)
            for ver in ("v3", "v4")
        }
        _op = dve_ops.DveOp("ADD_WRAP_EWALD", _spec, subdim=False, uops_sha=_shas)
        dve_ops.OPS.append(_op)
        dve_ops._SUB_OPCODE_FOR_NAME[_op.name] = (
            dve_ops._CUSTOM_DVE_ROW_BASE + len(dve_ops.OPS) - 1
        )
        dve_ops.CUSTOM_DVE_SPECS[_op.name] = _spec
        dve_ops.ADD_WRAP_EWALD = _op
    AW = dve_ops.ADD_WRAP_EWALD

    tile.TileContext._drain_and_barrier = _cheap_drain_and_barrier
    nc = bacc.Bacc(None, target_bir_lowering=False)

    ftab = nc.dram_tensor("ftab", [128, FTAB_COLS], f32, kind="ExternalInput")
    lhs = nc.dram_tensor("lhs", [128, LHS_COLS], f32r, kind="ExternalInput")
    sout = nc.dram_tensor("sout", [2 * NH, OUT_COLS], f32, kind="ExternalOutput")

    with tile.TileContext(nc) as tc:
        CC = NH + 2 * NJ                                # 53 cols per chunk
        with (
            tc.tile_pool(name="const", bufs=1) as cp,
            tc.tile_pool(name="work", bufs=3) as wp,
            tc.tile_pool(name="psum", bufs=1, space="PSUM") as pp,
        ):
            ft = cp.tile([128, FTAB_COLS], f32)
            # split so the first chunks land (and unblock the DVE) early;
            # both pieces go on sync's HWDGE ring: same-ring pieces still
            # pipeline, and a scalar-ring DMA would force a second
            # ACT_TABLE_LOAD (~1.3us) into the scalar program.
            nc.sync.dma_start(out=ft[:, 0 : 3 * CC], in_=ftab[:, 0 : 3 * CC])
            nc.sync.dma_start(
                out=ft[:, 3 * CC : FTAB_COLS], in_=ftab[:, 3 * CC : FTAB_COLS]
            )
            lt = cp.tile([128, LHS_COLS], f32r)
            nc.gpsimd.dma_start(out=lt[:], in_=lhs[:])

            ps = pp.tile([2 * NH, OUT_COLS], f32)

            def pair_args(t):
                # pair col = bj*11 + j2, bj = b*21 + j3 (b=0 sin, b=1 cos)
                f2 = ft[:, CC * t : CC * t + NH]
                f3x = ft[:, CC * t + NH : CC * (t + 1)]
                return (
                    f2.unsqueeze(1).broadcast_to([128, 2 * NJ, NH]),
                    f3x.unsqueeze(2).broadcast_to([128, 2 * NJ, NH]),
                )

            # groups of chunks sharing one ACT op: pairs up front for
            # fewer/larger Sin calls, singles at the end so the pipeline
            # tail drains chunk-at-a-time.
            groups = [(0, 1), (2, 3), (4, 5), (6,), (7,)]
            for grp in groups:
                V = wp.tile([128, len(grp) * OUT_COLS], f32)
                for h, t in enumerate(grp):
                    in0, in1 = pair_args(t)
                    nc.vector._custom_dve(
                        AW,
                        out=V[:, OUT_COLS * h : OUT_COLS * (h + 1)].rearrange(
                            "p (bj j2) -> p bj j2", j2=NH
                        ),
                        in0=in0, in1=in1, s0=0.0, s1=0.5,
                    )
                AA = wp.tile([128, len(grp) * OUT_COLS], f32r)
                nc.scalar.activation(
                    out=AA[:], in_=V[:], func=Act.Sin, bias=0.0, scale=-TWOPI
                )
                for h, t in enumerate(grp):
                    nc.tensor.matmul(
                        out=ps[:],
                        lhsT=lt[:, 2 * NH * t : 2 * NH * (t + 1)],
                        rhs=AA[:, OUT_COLS * h : OUT_COLS * (h + 1)],
                        start=(t == 0), stop=(t == CHUNKS - 1),
                    )

            so = wp.tile([2 * NH, OUT_COLS], f32)
            nc.vector.tensor_copy(out=so[:], in_=ps[:])
            nc.sync.dma_start(out=sout[:], in_=so[:])

    nc.compile()
    return nc


def _get_nc():
    if "nc" not in _CACHE:
        _CACHE["nc"] = _build_nc()
    return _CACHE["nc"]


def _chunk_major(x, w):
    """atom a = t*128 + p  ->  [p, t*w + j]"""
    return x.reshape(CHUNKS, 128, w).transpose(1, 0, 2).reshape(128, CHUNKS * w)


def _host_inputs(q, r, cell):
    jj = np.arange(0, NH, dtype=np.float64)
    j3 = np.arange(-NK, NK + 1, dtype=np.float64)
    maps = []
    for c in range(N_CORES):
        b = c // CORES_PER_SYS
        half = c % CORES_PER_SYS
        lo = b * N_PER + half * ATOMS_PER_CORE
        rs = r[lo : lo + ATOMS_PER_CORE].astype(np.float64)
        qs = q[lo : lo + ATOMS_PER_CORE, 0].astype(np.float64)
        minv = np.linalg.inv(cell[b].astype(np.float64))
        phi = (rs @ minv) % 1.0
        phi_p = np.zeros((PADN, 3)); phi_p[:ATOMS_PER_CORE] = phi
        q_p = np.zeros(PADN); q_p[:ATOMS_PER_CORE] = qs
        ang1 = TWOPI * np.outer(phi_p[:, 0], jj)
        lhs = np.concatenate(
            [np.cos(ang1) * q_p[:, None], np.sin(ang1) * q_p[:, None]], axis=1
        )
        t2 = np.outer(phi_p[:, 1], jj); F2 = t2 - np.round(t2)
        t3 = np.outer(phi_p[:, 2], j3); F3 = t3 - np.round(t3)
        # per-chunk groups of 53 cols: F2 (11) | F3 (21) | F3 - 1/4 (21)
        ftab = _chunk_major(
            np.concatenate([F2, F3, F3 - 0.25], axis=1), NH + 2 * NJ
        ).astype(np.float32)
        maps.append({"ftab": ftab, "lhs": _chunk_major(lhs, 2 * NH).astype(np.float32)})
    return maps


def _host_weights(cell):
    """w[b, n1(0..10), n2(-10..10), n3(-10..10)] mirroring the reference."""
    k_sq_max = (TWOPI / DL) ** 2
    sigma_sq_half = SIGMA ** 2 / 2.0
    rng = np.arange(-NK, NK + 1, dtype=np.float64)
    n1, n2, n3 = np.meshgrid(rng[NK:], rng, rng, indexing="ij")
    nvec = np.stack([n1.ravel(), n2.ravel(), n3.ravel()], axis=1)
    hemi = (
        (nvec[:, 0] > 0)
        | ((nvec[:, 0] == 0) & (nvec[:, 1] > 0))
        | ((nvec[:, 0] == 0) & (nvec[:, 1] == 0) & (nvec[:, 2] > 0))
    )
    ws = []
    for b in range(B):
        cb = cell[b].astype(np.float64)
        G = TWOPI * np.linalg.inv(cb).T
        kvec = nvec @ G
        k_sq = np.sum(kvec ** 2, axis=1)
        mask = (k_sq > 0) & (k_sq <= k_sq_max) & hemi
        kfac = np.exp(-sigma_sq_half * k_sq) / (k_sq + EPS)
        vol = np.linalg.det(cb)
        ws.append(np.where(mask, 2.0 * kfac, 0.0) / vol)
    return np.stack(ws).reshape(B, NH, NJ, NJ)


def kernel(q, r, cell, batch):
    from concourse.bass_utils import run_bass_kernel_spmd

    q = np.asarray(q)
    r = np.asarray(r)
    cell = np.asarray(cell)

    nc = _get_nc()
    in_maps = _host_inputs(q, r, cell)
    res = run_bass_kernel_spmd(nc, in_maps, core_ids=list(range(N_CORES))).results

    w = _host_weights(cell)
    pot = np.zeros(B, np.float64)
    for b in range(B):
        M = (
            res[b * CORES_PER_SYS]["sout"].astype(np.float64)
            + res[b * CORES_PER_SYS + 1]["sout"].astype(np.float64)
        )
        # pair blocks are [n3, n2]-ordered: col = j3*11 + n2
        Crs = -M[0:NH, 0:NRECT].reshape(NH, NJ, NH)       # sum q c1 sin(th23)
        Css = -M[NH : 2 * NH, 0:NRECT].reshape(NH, NJ, NH)
        Crc = M[0:NH, NRECT:OUT_COLS].reshape(NH, NJ, NH)  # sum q c1 cos(th23)
        Csc = M[NH : 2 * NH, NRECT:OUT_COLS].reshape(NH, NJ, NH)
        wb = w[b]
        w_dir = wb[:, NK:, :].transpose(0, 2, 1)  # rect pair (n2, n3) itself
        w_mir = wb[:, NK::-1, ::-1].copy()        # its mirror (-n2, -n3)
        w_mir[:, 0, :] = 0.0                      # n2=0 row counted once
        w_mir = w_mir.transpose(0, 2, 1)
        s_sq_dir = (Crc - Css) ** 2 + (Crs + Csc) ** 2
        s_sq_mir = (Crc + Css) ** 2 + (Csc - Crs) ** 2
        recip = np.sum(w_dir * s_sq_dir) + np.sum(w_mir * s_sq_mir)
        qb = q[b * N_PER : (b + 1) * N_PER, 0].astype(np.float64)
        self_e = np.sum(qb ** 2) / (SIGMA * TWOPI ** 1.5)
        pot[b] = (recip - self_e) * NORM
    return pot.astype(np.float32)
